# revision 11
# baseline (speedup 1.0000x reference)
"""Gated Linear Attention on 8 Trainium2 NeuronCores.

Sharding: one (batch, head) pair per core (B=2 x H=4 = 8 cores). Each core
computes its head's full pipeline and emits a partial [N, D] output (bf16);
the host sums the 4 head partials per batch in f32.

v4 design:
  * All heavy matmuls in bf16 (1 PE cycle/row vs 4 for fp32); PSUM accums f32.
  * Per-chunk LOCAL decay (no global cumsum carry chain): within chunk c,
    b = L^T g'' (local inclusive cumsum). q~=q*exp(-b/16), k~=k*exp(+b/16);
    cross-chunk state rescaled once per chunk by the per-feature factor
    f = exp(-b_last/16) = ET[:,last]:  W_c = diag(f) (W_{c-1} + k~^T v).
    Local exponent args <= ~6, safe in bf16/f32.
  * z-projection folded into the main projection blob. Projection emission is
    split bank1-first (gate|z) so the per-chunk softplus (2 ACT + 1 DVE ops,
    clamp folded into ln via min(u,e^48)) overlaps the qkv matmuls and the
    L-matmul never stalls the PE.
  * ACT table discipline: exp+ln resolve to the combined table by blanking
    the exp-only/ln-only sets for the load-insertion pass (ids still index
    the real act_info.json). Silu via tanh in the final phase. 2 loads total.
  * Engine balance: Pool/GpSimd takes the psum->sbuf eviction copies
    (at-mask, state, ssq, oT); per-queue semaphore overhead (~250ns/op on
    ACT/DVE) is minimized by merging adjacent-column copies (q|k, v|gate)
    and batching phase-D tanh over all chunks in one instruction.
  * RMS r deferred and folded into the silu gate; bf16 I/O; contiguous 2KB+
    DMA rows; DMA descriptor issues spread across idle engine queues.
"""

import os
from contextlib import ExitStack

import numpy as np
import ml_dtypes

import concourse.bass as bass
import concourse.tile as tile
from concourse import bacc, mybir
from concourse.tile_rust import add_dep_helper
from concourse.bass_utils import run_bass_kernel_spmd

F32 = mybir.dt.float32
BF16 = mybir.dt.bfloat16
AF = mybir.ActivationFunctionType
ALU = mybir.AluOpType

B, N, D, H = 2, 1024, 1024, 4
KD, VD, DK, DV = 512, 1024, 128, 256
C = 128                    # chunk length (= token partitions)
NCH = N // C               # 8 chunks
NK = D // 128              # 8 contraction tiles
BLOBW = 896                # blob cols: q128 | k128 | v256 | gate256 | z128
GLN = 16.0
EPS = 1e-5
E48 = float(np.exp(48.0).astype(np.float32))

# module-level stash so test.py can grab profiling results
LAST_RESULTS = None

_BLANK_TABLES = ("exp_and_others", "natural_log", "exp_and_friends")
_tables_patched = False


def _patch_act_tables():
    """Steer the ACT-table-load chooser toward natural_log_exp_and_others so
    exp+ln never alternate table loads. Only the (name -> funcs) map used by
    the load-insertion pass and CoreSim is filtered; emitted act_func_set_ids
    still index the real act_info.json, so walrus/hardware see valid sets."""
    global _tables_patched
    if _tables_patched:
        return
    _tables_patched = True
    from concourse import hw_specs, bass_interp
    orig = hw_specs.get_activation_tables

    def patched(arch):
        tabs = dict(orig(arch))
        for name in _BLANK_TABLES:
            if name in tabs:
                tabs[name] = set()
        return tabs

    bacc.get_activation_tables = patched
    bass_interp.get_activation_tables = patched


def _emit_kernel(ctx: ExitStack, tc: "tile.TileContext", ap: dict):
    nc = tc.nc

    # Chain all PE instructions in program order (PE executes in-order; this
    # keeps the Tile scheduler from reordering matmuls within a PSUM bank,
    # which would break has_written clear ordering).
    pe_prev = [None]

    def mm(*args, **kw):
        inst = nc.tensor.matmul(*args, **kw)
        if pe_prev[0] is not None:
            add_dep_helper(inst.ins, pe_prev[0], sync=False, reason="pe-order")
        pe_prev[0] = inst.ins
        return inst

    def tr_(out, in_, ident):
        inst = nc.tensor.transpose(out, in_, ident)
        if pe_prev[0] is not None:
            add_dep_helper(inst.ins, pe_prev[0], sync=False, reason="pe-order")
        pe_prev[0] = inst.ins
        return inst

    xT, wblob, woutT = ap["xT"], ap["wblob"], ap["woutT"]
    bgk2, lmask, lmaskb = ap["bgk2"], ap["lmask"], ap["lmaskb"]
    ident32, identb = ap["ident32"], ap["identb"]
    out = ap["out"]

    consts = ctx.enter_context(tc.tile_pool(name="consts", bufs=1))
    wpool = ctx.enter_context(tc.tile_pool(name="wpool", bufs=1))
    work = ctx.enter_context(tc.tile_pool(name="work", bufs=2))
    wst = ctx.enter_context(tc.tile_pool(name="wst", bufs=2))
    store = ctx.enter_context(tc.tile_pool(name="store", bufs=1))
    outp = ctx.enter_context(tc.tile_pool(name="outp", bufs=2))
    ppool = ctx.enter_context(tc.tile_pool(name="ppool", bufs=2, space="PSUM"))
    ptr = ctx.enter_context(tc.tile_pool(name="ptr", bufs=2, space="PSUM"))
    pao = ctx.enter_context(tc.tile_pool(name="pao", bufs=1, space="PSUM"))
    pst = ctx.enter_context(tc.tile_pool(name="pst", bufs=1, space="PSUM"))

    # ---- weights + x (bf16): x chunk 0 first (gates the first matmul), blob
    # on the gpsimd queue, rest of x on sync, consts on vector. Each
    # dma_start costs ~600ns of issue time on its queue, so spread them.
    xsb = wpool.tile([128, N, NK], BF16)
    nc.sync.dma_start(out=xsb[:, 0:C, :], in_=xT[:, 0:C, :])
    wb_sb = wpool.tile([128, NK, BLOBW], BF16)
    for k in range(NK):
        nc.gpsimd.dma_start(out=wb_sb[:, k, :], in_=wblob[k])
    for c in range(1, NCH):
        nc.sync.dma_start(out=xsb[:, c * C:(c + 1) * C, :],
                          in_=xT[:, c * C:(c + 1) * C, :])
    wout_sb = wpool.tile([128, 2, D], BF16)
    for j in range(2):
        nc.gpsimd.dma_start(out=wout_sb[:, j, :], in_=woutT[j])

    # ---- constants (issued on the scalar queue) ----
    L_sb = consts.tile([128, 128], F32)          # L[s,t]=1 iff s<=t (triu)
    nc.scalar.dma_start(out=L_sb[:], in_=lmask[:])
    Lb_sb = consts.tile([128, 128], BF16)
    nc.scalar.dma_start(out=Lb_sb[:], in_=lmaskb[:])
    id_sb = consts.tile([128, 128], F32)
    nc.scalar.dma_start(out=id_sb[:], in_=ident32[:])
    idb_sb = consts.tile([128, 128], BF16)
    nc.scalar.dma_start(out=idb_sb[:], in_=identb[:])
    bg_sb = consts.tile([1, 128], BF16)
    nc.scalar.dma_start(out=bg_sb[:], in_=bgk2[:])
    ones_row = consts.tile([1, 128], BF16)
    nc.vector.memset(ones_row[:], 1.0)
    ones_col = consts.tile([128, 1], BF16)
    nc.vector.memset(ones_col[:], 1.0)
    eps_sb = consts.tile([128, 1], F32)
    nc.vector.memset(eps_sb[:], EPS)

    # persistent stores for the deferred gate/output phase
    vug_all = store.tile([128, NCH, 512], BF16)   # per chunk: v 0:256|gate 256:512
    oT_all = store.tile([128, NCH, DV], BF16)
    ssq_all = store.tile([128, NCH], F32)

    # ---- main loop ---------------------------------------------------------
    # proj psum [128,1024]: bank0 {q 0:128 | k 128:256 | v 256:512}
    # bank1 {gate 512:768 | z 768:896 | b_loc 896:1024}. bank1 (and its bias
    # close) is emitted BEFORE bank0 so softplus overlaps the qkv matmuls.
    # The L-matmul (emitted after the previous chunk's smalls) writes b_loc
    # into the start-cleared region via skip_group_check.
    def emit_proj(c):
        proj = ppool.tile([128, 1024], F32, tag="proj")
        tok = slice(c * C, (c + 1) * C)
        for k in range(NK):
            mm(proj[:, 512:896], lhsT=xsb[:, tok, k], rhs=wb_sb[:, k, 512:896],
               start=(k == 0), stop=False)
        bias_mm = mm(proj[:, 768:896], lhsT=ones_row[:], rhs=bg_sb[:],
                     start=False, stop=True)
        # softplus: g = ln(min(1+exp(-z), e^48))  (clamp folded into the ln)
        e1 = work.tile([128, 128], F32, tag="e1")
        i = nc.scalar.activation(e1[:], proj[:, 768:896], AF.Exp, scale=-1.0)
        add_dep_helper(i.ins, bias_mm.ins, sync=False, reason="z after close")
        u1 = work.tile([128, 128], F32, tag="u1")
        nc.vector.tensor_scalar(u1[:], e1[:], 1.0, E48, ALU.add, ALU.min)
        g_c = work.tile([128, 128], BF16, tag="g")
        nc.scalar.activation(g_c[:], u1[:], AF.Ln)
        for k in range(NK):
            mm(proj[:, 0:512], lhsT=xsb[:, tok, k], rhs=wb_sb[:, k, 0:512],
               start=(k == 0), stop=(k == NK - 1))
        return proj, g_c

    def emit_lmm(proj, g_c):
        return mm(proj[:, 896:1024], lhsT=Lb_sb[:], rhs=g_c[:],
                  start=False, stop=False, skip_group_check=True)

    state = {"w_prev": None}

    def emit_smalls(c, proj, lmm):
        # evictions + decay factors
        b_sb = work.tile([128, 128], F32, tag="b")
        i = nc.vector.tensor_copy(b_sb[:], proj[:, 896:1024])
        add_dep_helper(i.ins, lmm.ins, sync=False, reason="b after L-mm")
        tr = ptr.tile([128, 512], F32, tag="tr")
        tr_(tr[:, 0:128], b_sb[:], id_sb[:])          # bT [feat, tok]
        En_tok = work.tile([128, 128], F32, tag="Ent")
        nc.scalar.activation(En_tok[:], b_sb[:], AF.Exp, scale=1.0 / GLN)
        ET = work.tile([128, 128], F32, tag="ET")
        nc.scalar.activation(ET[:], tr[:, 0:128], AF.Exp, scale=-1.0 / GLN)
        EnT = work.tile([128, 128], F32, tag="EnT")
        nc.scalar.activation(EnT[:], tr[:, 0:128], AF.Exp, scale=1.0 / GLN)
        f_vec = ET[:, 127:128]                        # exp(-b_last/16) per feat

        qk_sb = work.tile([128, 256], F32, tag="qk")
        nc.vector.tensor_copy(qk_sb[:], proj[:, 0:256])
        tr_(tr[:, 128:256], qk_sb[:, 0:128], id_sb[:])
        qtT = work.tile([128, 128], BF16, tag="qtT")
        nc.vector.tensor_mul(qtT[:], tr[:, 128:256], ET[:])
        tr_(tr[:, 256:384], qk_sb[:, 128:256], id_sb[:])
        ktT = work.tile([128, 128], BF16, tag="ktT")
        nc.vector.tensor_mul(ktT[:], tr[:, 256:384], EnT[:])
        kt_tm = work.tile([128, 128], BF16, tag="kt")
        nc.gpsimd.tensor_mul(kt_tm[:], qk_sb[:, 128:256], En_tok[:])

        # v | gate eviction in one copy into the persistent store
        nc.scalar.copy(vug_all[:, c, :], proj[:, 256:768])
        v_tm = vug_all[:, c, 0:256]

        # intra-chunk attention: AT[s,t] masked s<=t
        ao = pao.tile([128, 512], F32, tag="ao")      # at 0:128|oT 128:384|ssq
        mm(ao[:, 0:128], lhsT=ktT[:], rhs=qtT[:], start=True, stop=True)
        at_m = work.tile([128, 128], BF16, tag="atm")
        nc.vector.tensor_mul(at_m[:], ao[:, 0:128], L_sb[:])

        # oT = W_prev^T q~^T + v^T AT  (two dv halves)
        w_prev = state["w_prev"]
        if c > 0:
            mm(ao[:, 128:256], lhsT=w_prev[:, 0:128], rhs=qtT[:],
               start=False, stop=False, skip_group_check=True)
            mm(ao[:, 256:384], lhsT=w_prev[:, 128:256], rhs=qtT[:],
               start=False, stop=False, skip_group_check=True)
        mm(ao[:, 128:256], lhsT=v_tm[:, 0:128], rhs=at_m[:],
           start=False, stop=False, skip_group_check=True)
        mm(ao[:, 256:384], lhsT=v_tm[:, 128:256], rhs=at_m[:],
           start=False, stop=False, skip_group_check=True)

        # state: W_c = diag(f) (W_{c-1} + k~^T v)
        st = pst.tile([128, DV], F32, tag="st")
        mm(st[:], lhsT=kt_tm[:], rhs=v_tm[:], start=True, stop=(c == 0))
        if c > 0:
            mm(st[:], lhsT=idb_sb[:], rhs=w_prev[:], start=False, stop=True)
        if c < NCH - 1:
            w_new = wst.tile([128, DV], BF16, tag="w")
            nc.vector.tensor_scalar(w_new[:], st[:], f_vec, None, ALU.mult)
            state["w_prev"] = w_new

        # ssq per token -> spare column of the at/ot bank, then to SBUF
        sq = work.tile([128, DV], BF16, tag="sq")
        nc.scalar.square(sq[:], ao[:, 128:384])
        mm(ao[:, 384:385], lhsT=sq[:, 0:128], rhs=ones_col[:],
           start=False, stop=False, skip_group_check=True)
        mm(ao[:, 384:385], lhsT=sq[:, 128:256], rhs=ones_col[:],
           start=False, stop=False, skip_group_check=True)
        nc.scalar.copy(ssq_all[:, c:c + 1], ao[:, 384:385])
        nc.vector.tensor_copy(oT_all[:, c, :], ao[:, 128:384])

    # software pipeline: proj(c+1) before smalls(c); L-mm(c+1) after smalls(c)
    proj0, g0 = emit_proj(0)
    lmm0 = emit_lmm(proj0, g0)
    cur = (proj0, lmm0)
    for c in range(NCH):
        if c + 1 < NCH:
            pj, gc = emit_proj(c + 1)
        emit_smalls(c, cur[0], cur[1])
        if c + 1 < NCH:
            cur = (pj, emit_lmm(pj, gc))

    # ---- Phase D: RMS scale, silu gate (via tanh), final projection --------
    s_sb = work.tile([128, 8], F32, tag="s")
    nc.scalar.activation(s_sb[:], ssq_all[:], AF.Ln, scale=1.0 / DV,
                         bias=eps_sb[:])
    r_all = work.tile([128, 8], F32, tag="r")
    r_ins = nc.scalar.activation(r_all[:], s_sb[:], AF.Exp, scale=-0.5)

    # all-chunk tanh in one ACT op (kept after the main loop's exp/ln so the
    # tanh table loads exactly once)
    th_all = store.tile([128, NCH, DV], F32)
    i = nc.scalar.activation(th_all[:], vug_all[:, :, 256:512], AF.Tanh,
                             scale=0.5)
    add_dep_helper(i.ins, r_ins.ins, sync=False, reason="tanh after r")
    thp_all = store.tile([128, NCH, DV], BF16)
    nc.vector.tensor_scalar(thp_all[:], th_all[:], 0.5, 0.5, ALU.mult, ALU.add)

    for c in range(NCH):
        tok = slice(c * C, (c + 1) * C)
        # gate = silu(ug) * r = (ug*r) * (0.5 + 0.5*tanh(ug/2))
        gate_tm = work.tile([128, DV], F32, tag="gate")
        nc.vector.scalar_tensor_tensor(gate_tm[:], vug_all[:, c, 256:512],
                                       r_all[:, c:c + 1], thp_all[:, c, :],
                                       ALU.mult, ALU.mult)
        tr2 = ptr.tile([128, 512], F32, tag="tr")
        tr_(tr2[:, 0:128], gate_tm[:, 0:128], id_sb[:])
        tr_(tr2[:, 128:256], gate_tm[:, 128:256], id_sb[:])
        gateT = work.tile([128, DV], BF16, tag="gT")
        nc.scalar.copy(gateT[:], tr2[:, 0:256])
        og = work.tile([128, DV], BF16, tag="og")
        nc.gpsimd.tensor_mul(og[:], oT_all[:, c, :], gateT[:])

        fin = ppool.tile([128, 1024], F32, tag="proj")
        for nb in range(2):
            cols = slice(nb * 512, (nb + 1) * 512)
            mm(fin[:, cols], lhsT=og[:, 0:128],
               rhs=wout_sb[:, 0, cols], start=True, stop=False)
            mm(fin[:, cols], lhsT=og[:, 128:256],
               rhs=wout_sb[:, 1, cols], start=False, stop=True)
        o_sb = outp.tile([128, 1024], BF16, tag="o")
        nc.scalar.copy(o_sb[:, 0:512], fin[:, 0:512])
        nc.vector.tensor_copy(o_sb[:, 512:1024], fin[:, 512:1024])
        nc.gpsimd.dma_start(out=out[tok, :], in_=o_sb[:])


def _build_nc():
    _patch_act_tables()
    nc = bacc.Bacc("TRN2", target_bir_lowering=False, debug=False, num_devices=8)
    ap = {
        "xT": nc.dram_tensor("xT", [128, N, NK], BF16, kind="ExternalInput").ap(),
        "wblob": nc.dram_tensor("wblob", [NK, 128, BLOBW], BF16,
                                kind="ExternalInput").ap(),
        "woutT": nc.dram_tensor("woutT", [2, 128, D], BF16,
                                kind="ExternalInput").ap(),
        "bgk2": nc.dram_tensor("bgk2", [1, 128], BF16, kind="ExternalInput").ap(),
        "lmask": nc.dram_tensor("lmask", [128, 128], F32,
                                kind="ExternalInput").ap(),
        "lmaskb": nc.dram_tensor("lmaskb", [128, 128], BF16,
                                 kind="ExternalInput").ap(),
        "ident32": nc.dram_tensor("ident32", [128, 128], F32,
                                  kind="ExternalInput").ap(),
        "identb": nc.dram_tensor("identb", [128, 128], BF16,
                                 kind="ExternalInput").ap(),
        "out": nc.dram_tensor("out", [N, D], BF16, kind="ExternalOutput").ap(),
    }
    with tile.TileContext(nc) as tc:
        with ExitStack() as ctx:
            _emit_kernel(ctx, tc, ap)
    nc.compile()
    return nc


def kernel(x, Wq, Wk, Wv, Wg, Wgk1, Wgk2, bgk2, Wout, rms_w):
    global LAST_RESULTS
    BF = ml_dtypes.bfloat16
    x = np.asarray(x, np.float32)
    Wz = (np.asarray(Wgk1, np.float32) @ np.asarray(Wgk2, np.float32))
    L = np.triu(np.ones((C, C), np.float32))
    I32 = np.eye(128, dtype=np.float32)

    in_maps = []
    for core in range(8):
        b, h = core // H, core % H
        xTb = np.ascontiguousarray(
            x[b].T.reshape(NK, 128, N).transpose(1, 2, 0)).astype(BF)
        blob = np.ascontiguousarray(np.concatenate([
            Wq[:, h * DK:(h + 1) * DK], Wk[:, h * DK:(h + 1) * DK],
            Wv[:, h * DV:(h + 1) * DV], Wg[:, h * DV:(h + 1) * DV],
            Wz[:, h * DK:(h + 1) * DK]],
            axis=1).astype(np.float32)).reshape(NK, 128, BLOBW).astype(BF)
        woutP = np.ascontiguousarray(
            (np.asarray(rms_w, np.float32)[:, None]
             * np.asarray(Wout, np.float32)[h * DV:(h + 1) * DV])
        ).reshape(2, 128, D).astype(BF)
        in_maps.append({
            "xT": xTb,
            "wblob": blob,
            "woutT": woutP,
            "bgk2": np.ascontiguousarray(
                np.asarray(bgk2, np.float32)[h * DK:(h + 1) * DK][None, :]
            ).astype(BF),
            "lmask": L,
            "lmaskb": L.astype(BF),
            "ident32": I32,
            "identb": I32.astype(BF),
        })

    nc = _build_nc()
    trace = os.environ.get("BASSGLA_TRACE", "0") == "1"
    res = run_bass_kernel_spmd(nc, in_maps, list(range(8)), trace=trace)
    LAST_RESULTS = res

    out = np.zeros((B, N, D), np.float32)
    for core in range(8):
        out[core // H] += np.asarray(res.results[core]["out"], np.float32)
    return out


# revision 12
# speedup vs baseline: 1.2067x; 1.2067x over previous
"""Gated Linear Attention on 8 Trainium2 NeuronCores.

Sharding: one (batch, head) pair per core (B=2 x H=4 = 8 cores). Each core
computes its head's full pipeline and emits a partial [N, D] output (bf16);
the host sums the 4 head partials per batch in f32.

v4 design:
  * All heavy matmuls in bf16 (1 PE cycle/row vs 4 for fp32); PSUM accums f32.
  * Per-chunk LOCAL decay (no global cumsum carry chain): within chunk c,
    b = L^T g'' (local inclusive cumsum). q~=q*exp(-b/16), k~=k*exp(+b/16);
    cross-chunk state rescaled once per chunk by the per-feature factor
    f = exp(-b_last/16) = ET[:,last]:  W_c = diag(f) (W_{c-1} + k~^T v).
    Local exponent args <= ~6, safe in bf16/f32.
  * z-projection folded into the main projection blob. Projection emission is
    split bank1-first (gate|z) so the per-chunk softplus (2 ACT + 1 DVE ops,
    clamp folded into ln via min(u,e^48)) overlaps the qkv matmuls and the
    L-matmul never stalls the PE.
  * ACT table discipline: exp+ln resolve to the combined table by blanking
    the exp-only/ln-only sets for the load-insertion pass (ids still index
    the real act_info.json). Silu via tanh in the final phase. 2 loads total.
  * Engine balance: Pool/GpSimd takes the psum->sbuf eviction copies
    (at-mask, state, ssq, oT); per-queue semaphore overhead (~250ns/op on
    ACT/DVE) is minimized by merging adjacent-column copies (q|k, v|gate)
    and batching phase-D tanh over all chunks in one instruction.
  * RMS r deferred and folded into the silu gate; bf16 I/O; contiguous 2KB+
    DMA rows; DMA descriptor issues spread across idle engine queues.
"""

import os
from contextlib import ExitStack

import numpy as np
import ml_dtypes

import concourse.bass as bass
import concourse.tile as tile
from concourse import bacc, mybir
from concourse.tile_rust import add_dep_helper
from concourse.bass_utils import run_bass_kernel_spmd

F32 = mybir.dt.float32
BF16 = mybir.dt.bfloat16
AF = mybir.ActivationFunctionType
ALU = mybir.AluOpType

B, N, D, H = 2, 1024, 1024, 4
KD, VD, DK, DV = 512, 1024, 128, 256
C = 128                    # chunk length (= token partitions)
NCH = N // C               # 8 chunks
NK = D // 128              # 8 contraction tiles
BLOBW = 896                # blob cols: q128 | k128 | v256 | gate256 | z128
GLN = 16.0
EPS = 1e-5
E48 = float(np.exp(48.0).astype(np.float32))

# module-level stash so test.py can grab profiling results
LAST_RESULTS = None

_BLANK_TABLES = ("exp_and_others", "natural_log", "exp_and_friends")
_tables_patched = False


def _patch_act_tables():
    """Steer the ACT-table-load chooser toward natural_log_exp_and_others so
    exp+ln never alternate table loads. Only the (name -> funcs) map used by
    the load-insertion pass and CoreSim is filtered; emitted act_func_set_ids
    still index the real act_info.json, so walrus/hardware see valid sets."""
    global _tables_patched
    if _tables_patched:
        return
    _tables_patched = True
    from concourse import hw_specs, bass_interp
    orig = hw_specs.get_activation_tables

    def patched(arch):
        tabs = dict(orig(arch))
        for name in _BLANK_TABLES:
            if name in tabs:
                tabs[name] = set()
        return tabs

    bacc.get_activation_tables = patched
    bass_interp.get_activation_tables = patched


def _emit_kernel(ctx: ExitStack, tc: "tile.TileContext", ap: dict):
    nc = tc.nc

    # Chain all PE instructions in program order (PE executes in-order; this
    # keeps the Tile scheduler from reordering matmuls within a PSUM bank,
    # which would break has_written clear ordering).
    pe_prev = [None]

    def mm(*args, **kw):
        inst = nc.tensor.matmul(*args, **kw)
        if pe_prev[0] is not None:
            add_dep_helper(inst.ins, pe_prev[0], sync=False, reason="pe-order")
        pe_prev[0] = inst.ins
        return inst

    def tr_(out, in_, ident):
        inst = nc.tensor.transpose(out, in_, ident)
        if pe_prev[0] is not None:
            add_dep_helper(inst.ins, pe_prev[0], sync=False, reason="pe-order")
        pe_prev[0] = inst.ins
        return inst

    xT, wblob, woutT = ap["xT"], ap["wblob"], ap["woutT"]
    bgk2, lmask, lmaskb = ap["bgk2"], ap["lmask"], ap["lmaskb"]
    ident32, identb = ap["ident32"], ap["identb"]
    out = ap["out"]

    consts = ctx.enter_context(tc.tile_pool(name="consts", bufs=1))
    wpool = ctx.enter_context(tc.tile_pool(name="wpool", bufs=1))
    work = ctx.enter_context(tc.tile_pool(name="work", bufs=2))
    wst = ctx.enter_context(tc.tile_pool(name="wst", bufs=2))
    store = ctx.enter_context(tc.tile_pool(name="store", bufs=1))
    outp = ctx.enter_context(tc.tile_pool(name="outp", bufs=2))
    ppool = ctx.enter_context(tc.tile_pool(name="ppool", bufs=2, space="PSUM"))
    ptr = ctx.enter_context(tc.tile_pool(name="ptr", bufs=2, space="PSUM"))
    pao = ctx.enter_context(tc.tile_pool(name="pao", bufs=1, space="PSUM"))
    pst = ctx.enter_context(tc.tile_pool(name="pst", bufs=1, space="PSUM"))

    # ---- weights + x (bf16): x chunk 0 first (gates the first matmul), blob
    # on the gpsimd queue, rest of x on sync, consts on vector. Each
    # dma_start costs ~600ns of issue time on its queue, so spread them.
    xsb = wpool.tile([128, N, NK], BF16)
    nc.sync.dma_start(out=xsb[:, 0:C, :], in_=xT[:, 0:C, :])
    wb_sb = wpool.tile([128, NK, BLOBW], BF16)
    for k in range(NK):
        nc.gpsimd.dma_start(out=wb_sb[:, k, :], in_=wblob[k])
    for c in range(1, NCH):
        nc.sync.dma_start(out=xsb[:, c * C:(c + 1) * C, :],
                          in_=xT[:, c * C:(c + 1) * C, :])
    wout_sb = wpool.tile([128, 2, D], BF16)
    for j in range(2):
        nc.gpsimd.dma_start(out=wout_sb[:, j, :], in_=woutT[j])

    # ---- constants (issued on the scalar queue) ----
    L_sb = consts.tile([128, 128], F32)          # L[s,t]=1 iff s<=t (triu)
    nc.scalar.dma_start(out=L_sb[:], in_=lmask[:])
    Lb_sb = consts.tile([128, 128], BF16)
    nc.scalar.dma_start(out=Lb_sb[:], in_=lmaskb[:])
    id_sb = consts.tile([128, 128], F32)
    nc.scalar.dma_start(out=id_sb[:], in_=ident32[:])
    idb_sb = consts.tile([128, 128], BF16)
    nc.scalar.dma_start(out=idb_sb[:], in_=identb[:])
    bg_sb = consts.tile([1, 128], BF16)
    nc.scalar.dma_start(out=bg_sb[:], in_=bgk2[:])
    ones_row = consts.tile([1, 128], BF16)
    nc.vector.memset(ones_row[:], 1.0)
    ones_col = consts.tile([128, 1], BF16)
    nc.vector.memset(ones_col[:], 1.0)
    eps_sb = consts.tile([128, 1], F32)
    nc.vector.memset(eps_sb[:], EPS)

    # persistent stores for the deferred gate/output phase
    vug_all = store.tile([128, NCH, 512], BF16)   # per chunk: v 0:256|gate 256:512
    oT_all = store.tile([128, NCH, DV], BF16)
    ssq_all = store.tile([128, NCH], F32)

    # ---- main loop ---------------------------------------------------------
    # proj psum [128,1024]: bank0 {q 0:128 | k 128:256 | v 256:512}
    # bank1 {gate 512:768 | z 768:896 | b_loc 896:1024}. bank1 (and its bias
    # close) is emitted BEFORE bank0 so softplus overlaps the qkv matmuls.
    # The L-matmul (emitted after the previous chunk's smalls) writes b_loc
    # into the start-cleared region via skip_group_check.
    def emit_proj(c):
        proj = ppool.tile([128, 1024], F32, tag="proj")
        tok = slice(c * C, (c + 1) * C)
        for k in range(NK):
            mm(proj[:, 512:896], lhsT=xsb[:, tok, k], rhs=wb_sb[:, k, 512:896],
               start=(k == 0), stop=False)
        bias_mm = mm(proj[:, 768:896], lhsT=ones_row[:], rhs=bg_sb[:],
                     start=False, stop=True)
        # softplus: g = ln(min(1+exp(-z), e^48))  (clamp folded into the ln)
        e1 = work.tile([128, 128], F32, tag="e1")
        i = nc.scalar.activation(e1[:], proj[:, 768:896], AF.Exp, scale=-1.0)
        add_dep_helper(i.ins, bias_mm.ins, sync=False, reason="z after close")
        u1 = work.tile([128, 128], F32, tag="u1")
        nc.vector.tensor_scalar(u1[:], e1[:], 1.0, E48, ALU.add, ALU.min)
        g_c = work.tile([128, 128], BF16, tag="g")
        nc.scalar.activation(g_c[:], u1[:], AF.Ln)
        for k in range(NK):
            mm(proj[:, 0:512], lhsT=xsb[:, tok, k], rhs=wb_sb[:, k, 0:512],
               start=(k == 0), stop=(k == NK - 1))
        return proj, g_c

    def emit_lmm(proj, g_c):
        return mm(proj[:, 896:1024], lhsT=Lb_sb[:], rhs=g_c[:],
                  start=False, stop=False, skip_group_check=True)

    state = {"w_prev": None}

    def emit_smalls(c, proj, lmm):
        # evictions + decay factors
        b_sb = work.tile([128, 128], F32, tag="b")
        i = nc.vector.tensor_copy(b_sb[:], proj[:, 896:1024])
        add_dep_helper(i.ins, lmm.ins, sync=False, reason="b after L-mm")
        tr = ptr.tile([128, 512], F32, tag="tr")
        tr_(tr[:, 0:128], b_sb[:], id_sb[:])          # bT [feat, tok]
        En_tok = work.tile([128, 128], F32, tag="Ent")
        nc.scalar.activation(En_tok[:], b_sb[:], AF.Exp, scale=1.0 / GLN)
        ET = work.tile([128, 128], F32, tag="ET")
        nc.scalar.activation(ET[:], tr[:, 0:128], AF.Exp, scale=-1.0 / GLN)
        EnT = work.tile([128, 128], F32, tag="EnT")
        nc.scalar.activation(EnT[:], tr[:, 0:128], AF.Exp, scale=1.0 / GLN)
        f_vec = ET[:, 127:128]                        # exp(-b_last/16) per feat

        qk_sb = work.tile([128, 256], F32, tag="qk")
        nc.vector.tensor_copy(qk_sb[:], proj[:, 0:256])
        tr_(tr[:, 128:256], qk_sb[:, 0:128], id_sb[:])
        qtT = work.tile([128, 128], BF16, tag="qtT")
        nc.vector.tensor_mul(qtT[:], tr[:, 128:256], ET[:])
        tr_(tr[:, 256:384], qk_sb[:, 128:256], id_sb[:])
        ktT = work.tile([128, 128], BF16, tag="ktT")
        nc.vector.tensor_mul(ktT[:], tr[:, 256:384], EnT[:])
        kt_tm = work.tile([128, 128], BF16, tag="kt")
        nc.vector.tensor_mul(kt_tm[:], qk_sb[:, 128:256], En_tok[:])

        # v | gate eviction in one copy into the persistent store
        nc.scalar.copy(vug_all[:, c, :], proj[:, 256:768])
        v_tm = vug_all[:, c, 0:256]

        # intra-chunk attention: AT[s,t] masked s<=t
        ao = pao.tile([128, 512], F32, tag="ao")      # at 0:128|oT 128:384|ssq
        mm(ao[:, 0:128], lhsT=ktT[:], rhs=qtT[:], start=True, stop=True)
        at_m = work.tile([128, 128], BF16, tag="atm")
        nc.vector.tensor_mul(at_m[:], ao[:, 0:128], L_sb[:])

        # oT = W_prev^T q~^T + v^T AT  (two dv halves)
        w_prev = state["w_prev"]
        if c > 0:
            mm(ao[:, 128:256], lhsT=w_prev[:, 0:128], rhs=qtT[:],
               start=False, stop=False, skip_group_check=True)
            mm(ao[:, 256:384], lhsT=w_prev[:, 128:256], rhs=qtT[:],
               start=False, stop=False, skip_group_check=True)
        mm(ao[:, 128:256], lhsT=v_tm[:, 0:128], rhs=at_m[:],
           start=False, stop=False, skip_group_check=True)
        mm(ao[:, 256:384], lhsT=v_tm[:, 128:256], rhs=at_m[:],
           start=False, stop=False, skip_group_check=True)

        # state: W_c = diag(f) (W_{c-1} + k~^T v)
        st = pst.tile([128, DV], F32, tag="st")
        mm(st[:], lhsT=kt_tm[:], rhs=v_tm[:], start=True, stop=(c == 0))
        if c > 0:
            mm(st[:], lhsT=idb_sb[:], rhs=w_prev[:], start=False, stop=True)
        if c < NCH - 1:
            w_new = wst.tile([128, DV], BF16, tag="w")
            nc.vector.tensor_scalar(w_new[:], st[:], f_vec, None, ALU.mult)
            state["w_prev"] = w_new

        # ssq per token -> spare column of the at/ot bank, then to SBUF
        sq = work.tile([128, DV], BF16, tag="sq")
        nc.scalar.square(sq[:], ao[:, 128:384])
        mm(ao[:, 384:385], lhsT=sq[:, 0:128], rhs=ones_col[:],
           start=False, stop=False, skip_group_check=True)
        mm(ao[:, 384:385], lhsT=sq[:, 128:256], rhs=ones_col[:],
           start=False, stop=False, skip_group_check=True)
        nc.scalar.copy(ssq_all[:, c:c + 1], ao[:, 384:385])
        nc.vector.tensor_copy(oT_all[:, c, :], ao[:, 128:384])

    # software pipeline: proj(c+1) before smalls(c); L-mm(c+1) after smalls(c)
    proj0, g0 = emit_proj(0)
    lmm0 = emit_lmm(proj0, g0)
    cur = (proj0, lmm0)
    for c in range(NCH):
        if c + 1 < NCH:
            pj, gc = emit_proj(c + 1)
        emit_smalls(c, cur[0], cur[1])
        if c + 1 < NCH:
            cur = (pj, emit_lmm(pj, gc))

    # ---- Phase D: RMS scale, silu gate (via tanh), final projection --------
    s_sb = work.tile([128, 8], F32, tag="s")
    nc.scalar.activation(s_sb[:], ssq_all[:], AF.Ln, scale=1.0 / DV,
                         bias=eps_sb[:])
    r_all = work.tile([128, 8], F32, tag="r")
    r_ins = nc.scalar.activation(r_all[:], s_sb[:], AF.Exp, scale=-0.5)

    # all-chunk tanh in one ACT op (kept after the main loop's exp/ln so the
    # tanh table loads exactly once)
    th_all = store.tile([128, NCH, DV], F32)
    i = nc.scalar.activation(th_all[:], vug_all[:, :, 256:512], AF.Tanh,
                             scale=0.5)
    add_dep_helper(i.ins, r_ins.ins, sync=False, reason="tanh after r")
    thp_all = store.tile([128, NCH, DV], BF16)
    nc.vector.tensor_scalar(thp_all[:], th_all[:], 0.5, 0.5, ALU.mult, ALU.add)

    for c in range(NCH):
        tok = slice(c * C, (c + 1) * C)
        # gate = silu(ug) * r = (ug*r) * (0.5 + 0.5*tanh(ug/2))
        gate_tm = work.tile([128, DV], F32, tag="gate")
        nc.vector.scalar_tensor_tensor(gate_tm[:], vug_all[:, c, 256:512],
                                       r_all[:, c:c + 1], thp_all[:, c, :],
                                       ALU.mult, ALU.mult)
        tr2 = ptr.tile([128, 512], F32, tag="tr")
        tr_(tr2[:, 0:128], gate_tm[:, 0:128], id_sb[:])
        tr_(tr2[:, 128:256], gate_tm[:, 128:256], id_sb[:])
        gateT = work.tile([128, DV], BF16, tag="gT")
        nc.scalar.copy(gateT[:], tr2[:, 0:256])
        og = work.tile([128, DV], BF16, tag="og")
        nc.vector.tensor_mul(og[:], oT_all[:, c, :], gateT[:])

        fin = ppool.tile([128, 1024], F32, tag="proj")
        for nb in range(2):
            cols = slice(nb * 512, (nb + 1) * 512)
            mm(fin[:, cols], lhsT=og[:, 0:128],
               rhs=wout_sb[:, 0, cols], start=True, stop=False)
            mm(fin[:, cols], lhsT=og[:, 128:256],
               rhs=wout_sb[:, 1, cols], start=False, stop=True)
        o_sb = outp.tile([128, 1024], BF16, tag="o")
        nc.scalar.copy(o_sb[:, 0:512], fin[:, 0:512])
        nc.vector.tensor_copy(o_sb[:, 512:1024], fin[:, 512:1024])
        nc.gpsimd.dma_start(out=out[tok, :], in_=o_sb[:])


def _build_nc():
    _patch_act_tables()
    nc = bacc.Bacc("TRN2", target_bir_lowering=False, debug=False, num_devices=8)
    ap = {
        "xT": nc.dram_tensor("xT", [128, N, NK], BF16, kind="ExternalInput").ap(),
        "wblob": nc.dram_tensor("wblob", [NK, 128, BLOBW], BF16,
                                kind="ExternalInput").ap(),
        "woutT": nc.dram_tensor("woutT", [2, 128, D], BF16,
                                kind="ExternalInput").ap(),
        "bgk2": nc.dram_tensor("bgk2", [1, 128], BF16, kind="ExternalInput").ap(),
        "lmask": nc.dram_tensor("lmask", [128, 128], F32,
                                kind="ExternalInput").ap(),
        "lmaskb": nc.dram_tensor("lmaskb", [128, 128], BF16,
                                 kind="ExternalInput").ap(),
        "ident32": nc.dram_tensor("ident32", [128, 128], F32,
                                  kind="ExternalInput").ap(),
        "identb": nc.dram_tensor("identb", [128, 128], BF16,
                                 kind="ExternalInput").ap(),
        "out": nc.dram_tensor("out", [N, D], BF16, kind="ExternalOutput").ap(),
    }
    with tile.TileContext(nc) as tc:
        with ExitStack() as ctx:
            _emit_kernel(ctx, tc, ap)
    nc.compile()
    return nc


def kernel(x, Wq, Wk, Wv, Wg, Wgk1, Wgk2, bgk2, Wout, rms_w):
    global LAST_RESULTS
    BF = ml_dtypes.bfloat16
    x = np.asarray(x, np.float32)
    Wz = (np.asarray(Wgk1, np.float32) @ np.asarray(Wgk2, np.float32))
    L = np.triu(np.ones((C, C), np.float32))
    I32 = np.eye(128, dtype=np.float32)

    in_maps = []
    for core in range(8):
        b, h = core // H, core % H
        xTb = np.ascontiguousarray(
            x[b].T.reshape(NK, 128, N).transpose(1, 2, 0)).astype(BF)
        blob = np.ascontiguousarray(np.concatenate([
            Wq[:, h * DK:(h + 1) * DK], Wk[:, h * DK:(h + 1) * DK],
            Wv[:, h * DV:(h + 1) * DV], Wg[:, h * DV:(h + 1) * DV],
            Wz[:, h * DK:(h + 1) * DK]],
            axis=1).astype(np.float32)).reshape(NK, 128, BLOBW).astype(BF)
        woutP = np.ascontiguousarray(
            (np.asarray(rms_w, np.float32)[:, None]
             * np.asarray(Wout, np.float32)[h * DV:(h + 1) * DV])
        ).reshape(2, 128, D).astype(BF)
        in_maps.append({
            "xT": xTb,
            "wblob": blob,
            "woutT": woutP,
            "bgk2": np.ascontiguousarray(
                np.asarray(bgk2, np.float32)[h * DK:(h + 1) * DK][None, :]
            ).astype(BF),
            "lmask": L,
            "lmaskb": L.astype(BF),
            "ident32": I32,
            "identb": I32.astype(BF),
        })

    nc = _build_nc()
    trace = os.environ.get("BASSGLA_TRACE", "0") == "1"
    res = run_bass_kernel_spmd(nc, in_maps, list(range(8)), trace=trace)
    LAST_RESULTS = res

    out = np.zeros((B, N, D), np.float32)
    for core in range(8):
        out[core // H] += np.asarray(res.results[core]["out"], np.float32)
    return out


# revision 13
# speedup vs baseline: 1.5456x; 1.2809x over previous
"""Gated Linear Attention on 8 Trainium2 NeuronCores.

Sharding: one (batch, head) pair per core (B=2 x H=4 = 8 cores). Each core
computes its head's full pipeline and emits a partial [N, D] output (bf16);
the host sums the 4 head partials per batch in f32.

v4 design:
  * All heavy matmuls in bf16 (1 PE cycle/row vs 4 for fp32); PSUM accums f32.
  * Per-chunk LOCAL decay (no global cumsum carry chain): within chunk c,
    b = L^T g'' (local inclusive cumsum). q~=q*exp(-b/16), k~=k*exp(+b/16);
    cross-chunk state rescaled once per chunk by the per-feature factor
    f = exp(-b_last/16) = ET[:,last]:  W_c = diag(f) (W_{c-1} + k~^T v).
    Local exponent args <= ~6, safe in bf16/f32.
  * z-projection folded into the main projection blob. Projection emission is
    split bank1-first (gate|z) so the per-chunk softplus (2 ACT + 1 DVE ops,
    clamp folded into ln via min(u,e^48)) overlaps the qkv matmuls and the
    L-matmul never stalls the PE.
  * ACT table discipline: exp+ln resolve to the combined table by blanking
    the exp-only/ln-only sets for the load-insertion pass (ids still index
    the real act_info.json). Silu via tanh in the final phase. 2 loads total.
  * Engine balance: Pool/GpSimd takes the psum->sbuf eviction copies
    (at-mask, state, ssq, oT); per-queue semaphore overhead (~250ns/op on
    ACT/DVE) is minimized by merging adjacent-column copies (q|k, v|gate)
    and batching phase-D tanh over all chunks in one instruction.
  * RMS r deferred and folded into the silu gate; bf16 I/O; contiguous 2KB+
    DMA rows; DMA descriptor issues spread across idle engine queues.
"""

import os
from contextlib import ExitStack

import numpy as np
import ml_dtypes

import concourse.bass as bass
import concourse.tile as tile
from concourse import bacc, mybir
from concourse.tile_rust import add_dep_helper
from concourse.bass_utils import run_bass_kernel_spmd

F32 = mybir.dt.float32
BF16 = mybir.dt.bfloat16
AF = mybir.ActivationFunctionType
ALU = mybir.AluOpType

B, N, D, H = 2, 1024, 1024, 4
KD, VD, DK, DV = 512, 1024, 128, 256
C = 128                    # chunk length (= token partitions)
NCH = N // C               # 8 chunks
NK = D // 128              # 8 contraction tiles
BLOBW = 896                # blob cols: q128 | k128 | v256 | gate256 | z128
GLN = 16.0
EPS = 1e-5
E48 = float(np.exp(48.0).astype(np.float32))

# module-level stash so test.py can grab profiling results
LAST_RESULTS = None

_BLANK_TABLES = ("exp_and_others", "natural_log", "exp_and_friends")
_tables_patched = False


def _patch_act_tables():
    """Steer the ACT-table-load chooser toward natural_log_exp_and_others so
    exp+ln never alternate table loads. Only the (name -> funcs) map used by
    the load-insertion pass and CoreSim is filtered; emitted act_func_set_ids
    still index the real act_info.json, so walrus/hardware see valid sets."""
    global _tables_patched
    if _tables_patched:
        return
    _tables_patched = True
    from concourse import hw_specs, bass_interp
    orig = hw_specs.get_activation_tables

    def patched(arch):
        tabs = dict(orig(arch))
        for name in _BLANK_TABLES:
            if name in tabs:
                tabs[name] = set()
        return tabs

    bacc.get_activation_tables = patched
    bass_interp.get_activation_tables = patched


def _emit_kernel(ctx: ExitStack, tc: "tile.TileContext", ap: dict):
    nc = tc.nc

    # Chain all PE instructions in program order (PE executes in-order; this
    # keeps the Tile scheduler from reordering matmuls within a PSUM bank,
    # which would break has_written clear ordering).
    pe_prev = [None]

    def mm(*args, **kw):
        inst = nc.tensor.matmul(*args, **kw)
        if pe_prev[0] is not None:
            add_dep_helper(inst.ins, pe_prev[0], sync=False, reason="pe-order")
        pe_prev[0] = inst.ins
        return inst

    def tr_(out, in_, ident):
        inst = nc.tensor.transpose(out, in_, ident)
        if pe_prev[0] is not None:
            add_dep_helper(inst.ins, pe_prev[0], sync=False, reason="pe-order")
        pe_prev[0] = inst.ins
        return inst

    xT, wblob, woutT = ap["xT"], ap["wblob"], ap["woutT"]
    bgk2, lmask, lmaskb = ap["bgk2"], ap["lmask"], ap["lmaskb"]
    ident32, identb = ap["ident32"], ap["identb"]
    out = ap["out"]

    consts = ctx.enter_context(tc.tile_pool(name="consts", bufs=1))
    wpool = ctx.enter_context(tc.tile_pool(name="wpool", bufs=1))
    work = ctx.enter_context(tc.tile_pool(name="work", bufs=2))
    wst = ctx.enter_context(tc.tile_pool(name="wst", bufs=2))
    store = ctx.enter_context(tc.tile_pool(name="store", bufs=1))
    outp = ctx.enter_context(tc.tile_pool(name="outp", bufs=2))
    ppool = ctx.enter_context(tc.tile_pool(name="ppool", bufs=2, space="PSUM"))
    ptr = ctx.enter_context(tc.tile_pool(name="ptr", bufs=2, space="PSUM"))
    pao = ctx.enter_context(tc.tile_pool(name="pao", bufs=1, space="PSUM"))
    pst = ctx.enter_context(tc.tile_pool(name="pst", bufs=1, space="PSUM"))

    # ---- weights + x (bf16): x chunk 0 first (gates the first matmul), blob
    # on the gpsimd queue, rest of x on sync, consts on vector. Each
    # dma_start costs ~600ns of issue time on its queue, so spread them.
    xsb = wpool.tile([128, N, NK], BF16)
    nc.sync.dma_start(out=xsb[:, 0:C, :], in_=xT[:, 0:C, :])
    wb_sb = wpool.tile([128, NK, BLOBW], BF16)
    for k in range(NK):
        nc.gpsimd.dma_start(out=wb_sb[:, k, :], in_=wblob[k])
    for c in range(1, NCH):
        nc.sync.dma_start(out=xsb[:, c * C:(c + 1) * C, :],
                          in_=xT[:, c * C:(c + 1) * C, :])
    wout_sb = wpool.tile([128, 2, D], BF16)
    for j in range(2):
        nc.gpsimd.dma_start(out=wout_sb[:, j, :], in_=woutT[j])

    # ---- constants (issued on the scalar queue) ----
    L_sb = consts.tile([128, 128], F32)          # L[s,t]=1 iff s<=t (triu)
    nc.scalar.dma_start(out=L_sb[:], in_=lmask[:])
    Lb_sb = consts.tile([128, 128], BF16)
    nc.scalar.dma_start(out=Lb_sb[:], in_=lmaskb[:])
    id_sb = consts.tile([128, 128], F32)
    nc.scalar.dma_start(out=id_sb[:], in_=ident32[:])
    idb_sb = consts.tile([128, 128], BF16)
    nc.scalar.dma_start(out=idb_sb[:], in_=identb[:])
    bg_sb = consts.tile([1, 128], BF16)
    nc.scalar.dma_start(out=bg_sb[:], in_=bgk2[:])
    ones_row = consts.tile([1, 128], BF16)
    nc.vector.memset(ones_row[:], 1.0)
    ones_col = consts.tile([128, 1], BF16)
    nc.vector.memset(ones_col[:], 1.0)
    eps_sb = consts.tile([128, 1], F32)
    nc.vector.memset(eps_sb[:], EPS)

    # persistent stores for the deferred gate/output phase
    vug_all = store.tile([128, NCH, 512], BF16)   # per chunk: v 0:256|gate 256:512
    oT_all = store.tile([128, NCH, DV], BF16)
    ssq_all = store.tile([128, NCH], F32)

    # ---- main loop ---------------------------------------------------------
    # proj psum [128,1024]: bank0 {q 0:128 | k 128:256 | v 256:512}
    # bank1 {gate 512:768 | z 768:896 | b_loc 896:1024}. bank1 (and its bias
    # close) is emitted BEFORE bank0 so softplus overlaps the qkv matmuls.
    # b (token-major) and bT (feature-major) are both produced directly by
    # matmuls against the triangular mask:  b = L^T g,  bT = g^T L  -- no
    # psum->sbuf copy or PE transpose on the decay path.
    # Emission is staged so each engine queue pops work in the order the
    # consumers need it (one-chunk software pipeline).

    def P1(c):
        proj = ppool.tile([128, 1024], F32, tag="proj")
        tok = slice(c * C, (c + 1) * C)
        for k in range(NK):
            mm(proj[:, 512:896], lhsT=xsb[:, tok, k], rhs=wb_sb[:, k, 512:896],
               start=(k == 0), stop=False)
        bias_mm = mm(proj[:, 768:896], lhsT=ones_row[:], rhs=bg_sb[:],
                     start=False, stop=True)
        # softplus part a: e1 = exp(-z)
        e1 = work.tile([128, 128], F32, tag="e1")
        i = nc.scalar.activation(e1[:], proj[:, 768:896], AF.Exp, scale=-1.0)
        add_dep_helper(i.ins, bias_mm.ins, sync=False, reason="z after close")
        return proj, e1

    def SPb(c, e1):
        u1 = work.tile([128, 128], F32, tag="u1")
        nc.vector.tensor_scalar(u1[:], e1[:], 1.0, E48, ALU.add, ALU.min)
        return u1

    def SPc(c, u1):
        g_c = work.tile([128, 128], BF16, tag="g")
        nc.scalar.activation(g_c[:], u1[:], AF.Ln)
        return g_c

    def P0(c, proj):
        tok = slice(c * C, (c + 1) * C)
        for k in range(NK):
            mm(proj[:, 0:512], lhsT=xsb[:, tok, k], rhs=wb_sb[:, k, 0:512],
               start=(k == 0), stop=(k == NK - 1))

    def Bmm(c, proj, g_c):
        # b (token-major) into proj bank1 spare region; bT into the tr bank
        bmm = mm(proj[:, 896:1024], lhsT=Lb_sb[:], rhs=g_c[:],
                 start=False, stop=False, skip_group_check=True)
        tr = ptr.tile([128, 512], F32, tag="tr")
        mm(tr[:, 0:128], lhsT=g_c[:], rhs=Lb_sb[:], start=True, stop=True)
        return tr, bmm

    def Ex(c, proj, tr, bmm):
        En_tok = work.tile([128, 128], F32, tag="Ent")
        i = nc.scalar.activation(En_tok[:], proj[:, 896:1024], AF.Exp,
                                 scale=1.0 / GLN)
        add_dep_helper(i.ins, bmm.ins, sync=False, reason="b after b-mm")
        ET = work.tile([128, 128], F32, tag="ET")
        nc.scalar.activation(ET[:], tr[:, 0:128], AF.Exp, scale=-1.0 / GLN)
        EnT = work.tile([128, 128], F32, tag="EnT")
        nc.scalar.activation(EnT[:], tr[:, 0:128], AF.Exp, scale=1.0 / GLN)
        return En_tok, ET, EnT

    def QK(c, proj):
        qk_sb = work.tile([128, 256], F32, tag="qk")
        nc.vector.tensor_copy(qk_sb[:], proj[:, 0:256])
        nc.scalar.copy(vug_all[:, c, :], proj[:, 256:768])
        return qk_sb

    def T(c, tr, qk_sb):
        tr_(tr[:, 128:256], qk_sb[:, 0:128], id_sb[:])
        tr_(tr[:, 256:384], qk_sb[:, 128:256], id_sb[:])

    def M(c, tr, qk_sb, En_tok, ET, EnT):
        qtT = work.tile([128, 128], BF16, tag="qtT")
        nc.vector.tensor_mul(qtT[:], tr[:, 128:256], ET[:])
        ktT = work.tile([128, 128], BF16, tag="ktT")
        nc.vector.tensor_mul(ktT[:], tr[:, 256:384], EnT[:])
        kt_tm = work.tile([128, 128], BF16, tag="kt")
        nc.vector.tensor_mul(kt_tm[:], qk_sb[:, 128:256], En_tok[:])
        return qtT, ktT, kt_tm

    def A(c, qtT, ktT):
        ao = pao.tile([128, 512], F32, tag="ao")
        mm(ao[:, 0:128], lhsT=ktT[:], rhs=qtT[:], start=True, stop=True)
        return ao

    def AM(c, ao):
        at_m = work.tile([128, 128], BF16, tag="atm")
        nc.vector.tensor_mul(at_m[:], ao[:, 0:128], L_sb[:])
        return at_m

    def ST(c, kt_tm, v_tm, ET):
        st = pst.tile([128, DV], F32, tag="st")
        mm(st[:], lhsT=kt_tm[:], rhs=v_tm[:], start=True, stop=(c == 0))
        if c > 0:
            mm(st[:], lhsT=idb_sb[:], rhs=state["w_prev"][:], start=False,
               stop=True)
        if c < NCH - 1:
            w_new = wst.tile([128, DV], BF16, tag="w")
            nc.vector.tensor_scalar(w_new[:], st[:], ET[:, 127:128], None,
                                    ALU.mult)
            state["w_prev"] = w_new

    def OT(c, ao, at_m, qtT, v_tm):
        if c > 0:
            w_prev = state["w_prev_for_o"]
            mm(ao[:, 128:256], lhsT=w_prev[:, 0:128], rhs=qtT[:],
               start=False, stop=False, skip_group_check=True)
            mm(ao[:, 256:384], lhsT=w_prev[:, 128:256], rhs=qtT[:],
               start=False, stop=False, skip_group_check=True)
        mm(ao[:, 128:256], lhsT=v_tm[:, 0:128], rhs=at_m[:],
           start=False, stop=False, skip_group_check=True)
        mm(ao[:, 256:384], lhsT=v_tm[:, 128:256], rhs=at_m[:],
           start=False, stop=False, skip_group_check=True)

    def SQ(c, ao):
        sq = work.tile([128, DV], BF16, tag="sq")
        nc.scalar.square(sq[:], ao[:, 128:384])
        nc.vector.tensor_copy(oT_all[:, c, :], ao[:, 128:384])
        return sq

    def SSQ(c, ao, sq):
        mm(ao[:, 384:385], lhsT=sq[:, 0:128], rhs=ones_col[:],
           start=False, stop=False, skip_group_check=True)
        mm(ao[:, 384:385], lhsT=sq[:, 128:256], rhs=ones_col[:],
           start=False, stop=False, skip_group_check=True)
        nc.scalar.copy(ssq_all[:, c:c + 1], ao[:, 384:385])

    # ---- pipeline driver ----
    state = {"w_prev": None, "w_prev_for_o": None}
    pend = {}   # per-chunk carried values

    proj0, e1_0 = P1(0)
    u1_0 = SPb(0, e1_0)
    g_0 = SPc(0, u1_0)
    P0(0, proj0)
    pend[0] = dict(proj=proj0, g=g_0)

    for c in range(NCH):
        p = pend[c]
        proj, g_c = p["proj"], p["g"]
        tr, bmm = Bmm(c, proj, g_c)
        En_tok, ET, EnT = Ex(c, proj, tr, bmm)
        qk_sb = QK(c, proj)
        v_tm = vug_all[:, c, 0:256]
        if c + 1 < NCH:
            projn, e1n = P1(c + 1)
        T(c, tr, qk_sb)
        qtT, ktT, kt_tm = M(c, tr, qk_sb, En_tok, ET, EnT)
        if c + 1 < NCH:
            u1n = SPb(c + 1, e1n)
            P0(c + 1, projn)
            gn = SPc(c + 1, u1n)
            pend[c + 1] = dict(proj=projn, g=gn)
        # previous chunk's deferred ssq (PE filler while at-mask lands)
        if c > 0:
            SSQ(c - 1, pend_ao["ao"], pend_ao["sq"])
        ao = A(c, qtT, ktT)
        at_m = AM(c, ao)
        state["w_prev_for_o"] = state["w_prev"]
        ST(c, kt_tm, v_tm, ET)
        OT(c, ao, at_m, qtT, v_tm)
        sq = SQ(c, ao)
        pend_ao = dict(ao=ao, sq=sq)
    SSQ(NCH - 1, pend_ao["ao"], pend_ao["sq"])

    # ---- Phase D: RMS scale, silu gate (via tanh), final projection --------
    s_sb = work.tile([128, 8], F32, tag="s")
    nc.scalar.activation(s_sb[:], ssq_all[:], AF.Ln, scale=1.0 / DV,
                         bias=eps_sb[:])
    r_all = work.tile([128, 8], F32, tag="r")
    r_ins = nc.scalar.activation(r_all[:], s_sb[:], AF.Exp, scale=-0.5)

    # gates for all chunks first (ACT/DVE pipeline), then one big og mul,
    # then a dense PE stream of transposes + final projections.
    gate_all = store.tile([128, NCH, DV], F32)
    for c in range(NCH):
        th = work.tile([128, DV], F32, tag="th")
        i = nc.scalar.activation(th[:], vug_all[:, c, 256:512], AF.Tanh,
                                 scale=0.5)
        # keep tanh after the main loop's exp/ln (single table switch)
        add_dep_helper(i.ins, r_ins.ins, sync=False, reason="tanh after r")
        thp = work.tile([128, DV], F32, tag="thp")
        nc.vector.tensor_scalar(thp[:], th[:], 0.5, 0.5, ALU.mult, ALU.add)
        # gate = silu(ug) * r = (ug*r) * (0.5 + 0.5*tanh(ug/2))
        nc.vector.scalar_tensor_tensor(gate_all[:, c, :], vug_all[:, c, 256:512],
                                       r_all[:, c:c + 1], thp[:],
                                       ALU.mult, ALU.mult)
    gateT_all = store.tile([128, NCH, DV], BF16)
    for c in range(NCH):
        tr2 = ptr.tile([128, 512], F32, tag="tr")
        tr_(tr2[:, 0:128], gate_all[:, c, 0:128], id_sb[:])
        tr_(tr2[:, 128:256], gate_all[:, c, 128:256], id_sb[:])
        nc.scalar.copy(gateT_all[:, c, :], tr2[:, 0:256])
    og_all = store.tile([128, NCH, DV], BF16)
    nc.vector.tensor_mul(og_all[:], oT_all[:], gateT_all[:])

    for c in range(NCH):
        tok = slice(c * C, (c + 1) * C)
        fin = ppool.tile([128, 1024], F32, tag="proj")
        for nb in range(2):
            cols = slice(nb * 512, (nb + 1) * 512)
            mm(fin[:, cols], lhsT=og_all[:, c, 0:128],
               rhs=wout_sb[:, 0, cols], start=True, stop=False)
            mm(fin[:, cols], lhsT=og_all[:, c, 128:256],
               rhs=wout_sb[:, 1, cols], start=False, stop=True)
        o_sb = outp.tile([128, 1024], BF16, tag="o")
        nc.scalar.copy(o_sb[:, 0:512], fin[:, 0:512])
        nc.vector.tensor_copy(o_sb[:, 512:1024], fin[:, 512:1024])
        nc.gpsimd.dma_start(out=out[tok, :], in_=o_sb[:])


def _build_nc():
    _patch_act_tables()
    nc = bacc.Bacc("TRN2", target_bir_lowering=False, debug=False, num_devices=8)
    ap = {
        "xT": nc.dram_tensor("xT", [128, N, NK], BF16, kind="ExternalInput").ap(),
        "wblob": nc.dram_tensor("wblob", [NK, 128, BLOBW], BF16,
                                kind="ExternalInput").ap(),
        "woutT": nc.dram_tensor("woutT", [2, 128, D], BF16,
                                kind="ExternalInput").ap(),
        "bgk2": nc.dram_tensor("bgk2", [1, 128], BF16, kind="ExternalInput").ap(),
        "lmask": nc.dram_tensor("lmask", [128, 128], F32,
                                kind="ExternalInput").ap(),
        "lmaskb": nc.dram_tensor("lmaskb", [128, 128], BF16,
                                 kind="ExternalInput").ap(),
        "ident32": nc.dram_tensor("ident32", [128, 128], F32,
                                  kind="ExternalInput").ap(),
        "identb": nc.dram_tensor("identb", [128, 128], BF16,
                                 kind="ExternalInput").ap(),
        "out": nc.dram_tensor("out", [N, D], BF16, kind="ExternalOutput").ap(),
    }
    with tile.TileContext(nc) as tc:
        with ExitStack() as ctx:
            _emit_kernel(ctx, tc, ap)
    nc.compile()
    return nc


def kernel(x, Wq, Wk, Wv, Wg, Wgk1, Wgk2, bgk2, Wout, rms_w):
    global LAST_RESULTS
    BF = ml_dtypes.bfloat16
    x = np.asarray(x, np.float32)
    Wz = (np.asarray(Wgk1, np.float32) @ np.asarray(Wgk2, np.float32))
    L = np.triu(np.ones((C, C), np.float32))
    I32 = np.eye(128, dtype=np.float32)

    in_maps = []
    for core in range(8):
        b, h = core // H, core % H
        xTb = np.ascontiguousarray(
            x[b].T.reshape(NK, 128, N).transpose(1, 2, 0)).astype(BF)
        blob = np.ascontiguousarray(np.concatenate([
            Wq[:, h * DK:(h + 1) * DK], Wk[:, h * DK:(h + 1) * DK],
            Wv[:, h * DV:(h + 1) * DV], Wg[:, h * DV:(h + 1) * DV],
            Wz[:, h * DK:(h + 1) * DK]],
            axis=1).astype(np.float32)).reshape(NK, 128, BLOBW).astype(BF)
        woutP = np.ascontiguousarray(
            (np.asarray(rms_w, np.float32)[:, None]
             * np.asarray(Wout, np.float32)[h * DV:(h + 1) * DV])
        ).reshape(2, 128, D).astype(BF)
        in_maps.append({
            "xT": xTb,
            "wblob": blob,
            "woutT": woutP,
            "bgk2": np.ascontiguousarray(
                np.asarray(bgk2, np.float32)[h * DK:(h + 1) * DK][None, :]
            ).astype(BF),
            "lmask": L,
            "lmaskb": L.astype(BF),
            "ident32": I32,
            "identb": I32.astype(BF),
        })

    nc = _build_nc()
    trace = os.environ.get("BASSGLA_TRACE", "0") == "1"
    res = run_bass_kernel_spmd(nc, in_maps, list(range(8)), trace=trace)
    LAST_RESULTS = res

    out = np.zeros((B, N, D), np.float32)
    for core in range(8):
        out[core // H] += np.asarray(res.results[core]["out"], np.float32)
    return out
